# revision 17
# baseline (speedup 1.0000x reference)
"""DotAttention Trainium2 Bass kernel.

reference:
    h = hidden[-1]                                  # [B, H]
    scores = einsum("bsh,bh->bs", enc, h)           # [B, S]
    scores = where(mask, scores, -1e9)
    attn = softmax(scores, axis=1)                  # [B, S]
    context = einsum("bs,bsh->bh", attn, enc)       # [B, H]
    return (context, attn)

Sharding: pure data parallel over B=32 across 8 cores (4 batches/core).

Per-core design (memory-bound: 64 MiB of encoder_outputs per core, read once):
  - s = p*32 + t index split: scores/mask/attn live as [128, 32] tiles with
    the sequence dim on partitions; no on-chip transposes are needed.
  - E streamed in [128, 4, 1024] tiles (2 MiB DMAs, 16 KiB contiguous per
    partition).
  - scores: fused DVE tensor_tensor_reduce (E*h_bcast, sum over free dim).
  - softmax: DVE row-reduce + gpsimd partition_all_reduce; exp on ACT with
    per-partition bias=-max and accum_out row sums.
  - context: PE matmul, attn column [128,1] stationary, E tile [128,512]
    moving, PSUM accumulation over the 32 s-column tiles.
"""

import numpy as np
from contextlib import ExitStack

import concourse.bass as bass
import concourse.bacc as bacc
import concourse.tile as tile
from concourse import mybir, bass_isa
from concourse.bass_utils import run_bass_kernel_spmd

NCORES = 8
B, S, H = 32, 4096, 1024
BPC = B // NCORES      # batches per core
P = 128                # partitions
T = S // P             # 32 score columns per batch
KT = 4                 # s-columns per E tile (2 MiB DMA granularity)
NK = T // KT           # E tiles per batch
NEG = -1.0e9
F32 = mybir.dt.float32

_cache = {}


def build_nc():
    nc = bacc.Bacc(None, target_bir_lowering=False)
    enc = nc.dram_tensor("enc", [BPC, S, H], F32, kind="ExternalInput")
    hv = nc.dram_tensor("hv", [BPC, H], F32, kind="ExternalInput")
    mk = nc.dram_tensor("mk", [BPC, S], mybir.dt.uint8, kind="ExternalInput")
    cto = nc.dram_tensor("cto", [BPC, H], F32, kind="ExternalOutput")
    ato = nc.dram_tensor("ato", [BPC, S], F32, kind="ExternalOutput")

    with tile.TileContext(nc) as tc, ExitStack() as ctx:
        epool = ctx.enter_context(tc.tile_pool(name="epool", bufs=10))
        hpool = ctx.enter_context(tc.tile_pool(name="hpool", bufs=2))
        spool = ctx.enter_context(tc.tile_pool(name="spool", bufs=3))
        small = ctx.enter_context(tc.tile_pool(name="small", bufs=2))
        singles = ctx.enter_context(tc.tile_pool(name="singles", bufs=1))
        psum = ctx.enter_context(tc.tile_pool(name="psum", bufs=4, space="PSUM"))
        outp = ctx.enter_context(tc.tile_pool(name="outp", bufs=2))

        negtile = singles.tile([P, T], F32)
        nc.vector.memset(negtile, NEG)
        ones1 = singles.tile([1, P], F32)
        nc.vector.memset(ones1, 1.0)
        nones1 = singles.tile([1, P], F32)
        nc.vector.memset(nones1, -1.0)

        for b in range(BPC):
            # h broadcast across partitions: [128, 1024]
            hb = hpool.tile([P, H], F32, tag="hb")
            nc.gpsimd.dma_start(out=hb, in_=hv[b : b + 1, :].to_broadcast([P, H]))
            # absorb the hb DMA wait on DVE so STT ops carry only their
            # e-tile wait (the STT ISA struct has few sync-wait slots)
            junk = small.tile([1, 1], F32, tag="junk")
            nc.vector.tensor_copy(out=junk, in_=hb[0:1, 0:1])

            mkt = small.tile([P, T], mybir.dt.uint8, tag="mask")
            nc.sync.dma_start(out=mkt, in_=mk[b].rearrange("(p t) -> p t", p=P))

            scr = small.tile([P, T], F32, tag="scores")
            encb = enc[b].rearrange("(p t) h -> p t h", p=P)  # [128, 32, 1024]

            etiles = []
            for k in range(NK):
                et = epool.tile([P, KT, H], F32, tag="e")
                nc.sync.dma_start(out=et, in_=encb[:, k * KT : (k + 1) * KT, :])
                etiles.append(et)
                for j in range(KT):
                    t = k * KT + j
                    prod = spool.tile([P, H], F32, tag="scratch")
                    nc.vector.tensor_mul(prod, et[:, j, :], hb)
                    # row-sum of products on ACT (accum_out), in-place copy
                    nc.scalar.activation(
                        out=prod,
                        in_=prod,
                        func=mybir.ActivationFunctionType.Copy,
                        bias=0.0,
                        scale=1.0,
                        accum_out=scr[:, t : t + 1],
                    )

            # masked scores
            sel = small.tile([P, T], F32, tag="sel")
            nc.vector.select(sel, mkt, scr, negtile)

            # global max over all 4096 scores
            rmax = small.tile([P, 1], F32, tag="rmax")
            nc.vector.tensor_reduce(
                out=rmax, in_=sel, axis=mybir.AxisListType.X, op=mybir.AluOpType.max
            )
            gmax1 = small.tile([1, 1], F32, tag="gmax1")
            nc.gpsimd.tensor_reduce(
                out=gmax1, in_=rmax, axis=mybir.AxisListType.C,
                op=mybir.AluOpType.max,
            )
            # broadcast -max to all partitions via PE: (-1s)^T @ max
            nmax_ps = psum.tile([P, 1], F32, tag="bc")
            nc.tensor.matmul(nmax_ps, nones1, gmax1)
            nmax = small.tile([P, 1], F32, tag="nmax")
            nc.scalar.copy(nmax, nmax_ps)

            # exp(x - max) with per-row partial sums accumulated on ACT
            probs = small.tile([P, T], F32, tag="probs")
            rsum = small.tile([P, 1], F32, tag="rsum")
            nc.scalar.activation(
                out=probs,
                in_=sel,
                func=mybir.ActivationFunctionType.Exp,
                bias=nmax,
                scale=1.0,
                accum_out=rsum,
            )
            gsum1 = small.tile([1, 1], F32, tag="gsum1")
            nc.gpsimd.tensor_reduce(
                out=gsum1, in_=rsum, axis=mybir.AxisListType.C,
                op=mybir.AluOpType.add,
            )
            inv1 = small.tile([1, 1], F32, tag="inv1")
            nc.vector.reciprocal(inv1, gsum1)
            inv_ps = psum.tile([P, 1], F32, tag="bc")
            nc.tensor.matmul(inv_ps, ones1, inv1)
            inv = small.tile([P, 1], F32, tag="inv")
            nc.scalar.copy(inv, inv_ps)

            attn = small.tile([P, T], F32, tag="attn")
            nc.vector.tensor_scalar_mul(attn, probs, inv)
            nc.sync.dma_start(out=ato[b].rearrange("(p t) -> p t", p=P), in_=attn)

            # context = sum_s attn[s] * E[s, :] on PE, accumulated in PSUM
            pc0 = psum.tile([1, 512], F32, tag="pc")
            pc1 = psum.tile([1, 512], F32, tag="pc")
            for t in range(T):
                k, j = divmod(t, KT)
                et = etiles[k]
                lhsT = attn[:, t : t + 1]
                nc.tensor.matmul(
                    pc0, lhsT, et[:, j, 0:512], start=(t == 0), stop=(t == T - 1)
                )
                nc.tensor.matmul(
                    pc1, lhsT, et[:, j, 512:1024], start=(t == 0), stop=(t == T - 1)
                )
            cs = outp.tile([1, H], F32, tag="ctx")
            nc.scalar.copy(cs[:, 0:512], pc0)
            nc.scalar.copy(cs[:, 512:1024], pc1)
            nc.sync.dma_start(out=cto[b : b + 1, :], in_=cs)

    nc.finalize()
    return nc


def _get_nc():
    if "nc" not in _cache:
        _cache["nc"] = build_nc()
    return _cache["nc"]


def run(hidden, encoder_outputs, mask, trace=False, **kwargs):
    """Shard, run on 8 cores, gather. Returns ((context, attn), BassKernelResults)."""
    h = np.ascontiguousarray(np.asarray(hidden, dtype=np.float32)[-1])  # [B, H]
    enc = np.ascontiguousarray(np.asarray(encoder_outputs, dtype=np.float32))
    mkf = np.ascontiguousarray(np.asarray(mask).astype(np.uint8))

    nc = _get_nc()
    in_maps = []
    for c in range(NCORES):
        lo, hi = c * BPC, (c + 1) * BPC
        in_maps.append(
            {
                "enc": enc[lo:hi],
                "hv": h[lo:hi],
                "mk": mkf[lo:hi],
            }
        )
    res = run_bass_kernel_spmd(
        nc, in_maps, core_ids=list(range(NCORES)), trace=trace, **kwargs
    )
    context = np.concatenate([r["cto"] for r in res.results], axis=0)
    attn = np.concatenate([r["ato"] for r in res.results], axis=0)
    return (context, attn), res


def kernel(hidden, encoder_outputs, mask, **_):
    out, _res = run(hidden, encoder_outputs, mask)
    return out
